# revision 3
# baseline (speedup 1.0000x reference)
"""Vocab-sharded AdaptiveSoftmax (log_softmax loss head) on 8 TRN2 NeuronCores.

Reference computes, for x:[2,512,1024] (flattened to T=1024 tokens, H=1024):
  head  = x @ W_head.T            -> [T, 20002]; cols 0:20000 raw logits, 20000:20002 cluster logits
  tail1 = cl0 + log_softmax(x @ W_proj1.T @ W_tail1.T)   -> [T, 40000]
  tail2 = cl1 + log_softmax(x @ W_proj2.T @ W_tail2.T)   -> [T, 140000]
  out   = concat([head[:, :20000], tail1, tail2], -1)    -> [T, 200000]

Sharding: vocab dim of head/tail weights split 8 ways (2500/5000/17500 rows per
core); x + projections replicated.  log_softmax normalizers need global
sum(exp(z)) over each tail's vocab -> AllReduce(add) of per-token sums.
The input data distribution keeps |logits| < ~2 so the max-subtraction in
log_softmax is unnecessary numerically; we all-reduce plain exp-sums.

Per-core kernel phases (all matmuls bf16 inputs, f32 PSUM accumulate):
  P : proj1T/proj2T = W_proj @ x.T   (kept in token-transposed layout for reuse
      as matmul lhsT), cluster logits per token.
  H : head raw logits -> out cols [0, 2500)        (weights streamed)
  T1: pass1 computes tail1 logits only to accumulate sum(exp()) per token,
      one AllReduce for all tokens; pass2 recomputes logits and writes
      logit + (cl0 - ln(gsum)) -> out cols [2500, 7500).
      (Recompute is cheaper than staging 10KB/partition of logits while the
      tail2 weights stream in.)
  T2: per 128-token tile: logits -> bf16 SBUF staging (double buffered) +
      exp-sum; per-tile AllReduce [128,1]; bias-add staged logits -> out cols
      [7500, 25000).  Collective latency hides under the next tile's matmuls.
"""

import sys

import numpy as np

if "/opt/trn_rl_repo" not in sys.path:
    sys.path.insert(0, "/opt/trn_rl_repo")

P = 128
T = 1024          # tokens (2*512)
NT = T // P       # 8 token tiles
H = 1024
KO_H = H // P     # 8
VH = 2500         # head vocab shard
V1 = 5000         # tail1 vocab shard
V2 = 17500        # tail2 vocab shard
E1, E2 = 512, 256
KO_1, KO_2 = E1 // P, E2 // P
C = 500           # free-dim chunk (<=512 psum bank)
NC_H, NC_1, NC_2 = VH // C, V1 // C, V2 // C
N_CORES = 8
VOUT = VH + V1 + V2   # 25000 per-core out cols

_CACHE = {}


def _build():
    import concourse.bacc as bacc
    import concourse.mybir as mybir
    import concourse.tile as tile
    from contextlib import ExitStack

    bf16 = mybir.dt.bfloat16
    f32 = mybir.dt.float32
    Exp = mybir.ActivationFunctionType.Exp
    Ident = mybir.ActivationFunctionType.Identity
    Ln = mybir.ActivationFunctionType.Ln

    nc = bacc.Bacc("TRN2", target_bir_lowering=False, debug=False,
                   num_devices=N_CORES)

    xT_d = nc.declare_dram_parameter("xT", [P, KO_H, T], bf16, False)
    whead_d = nc.declare_dram_parameter("wheadT", [P, KO_H, VH], bf16, False)
    wcl_d = nc.declare_dram_parameter("wclT", [P, KO_H, 2], bf16, False)
    wp1_d = nc.declare_dram_parameter("wp1T", [P, KO_H, E1], bf16, False)
    wp2_d = nc.declare_dram_parameter("wp2T", [P, KO_H, E2], bf16, False)
    wt1_d = nc.declare_dram_parameter("wt1T", [P, KO_1, V1], bf16, False)
    wt2_d = nc.declare_dram_parameter("wt2T", [P, KO_2, V2], bf16, False)
    out_d = nc.declare_dram_parameter("out", [T, VOUT], f32, True)

    # DRAM view with token tile split: [p, t_tile, vocab]
    out_r = out_d.ap().rearrange("(t p) v -> p t v", p=P)
    rg = [list(range(N_CORES))]

    with tile.TileContext(nc) as tc:
        with ExitStack() as root:
            pers = root.enter_context(tc.tile_pool(name="pers", bufs=1))
            psum = root.enter_context(
                tc.tile_pool(name="psum", bufs=7, space="PSUM"))
            dram = root.enter_context(
                tc.tile_pool(name="dram", bufs=1, space="DRAM"))
            scratch = root.enter_context(tc.tile_pool(name="scratch", bufs=4))
            outp = root.enter_context(tc.tile_pool(name="outp", bufs=3))

            # ---- persistent small tiles ----
            p1T = pers.tile([P, KO_1, T], bf16, name="p1T")
            p2T = pers.tile([P, KO_2, T], bf16, name="p2T")
            cl = pers.tile([P, NT, 2], f32, name="cl")
            s1acc = pers.tile([P, NT, NC_1], f32, name="s1acc")
            s1 = pers.tile([P, NT], f32, name="s1")
            g1 = pers.tile([P, NT], f32, name="g1")
            b1 = pers.tile([P, NT], f32, name="b1")

            # collective bounce buffers (internal DRAM)
            cc1_in = dram.tile([P, NT], f32, name="cc1_in")
            cc1_out = dram.tile([P, NT], f32, name="cc1_out",
                                addr_space="Shared")
            cc2_in = [dram.tile([P, 1], f32, name=f"cc2_in{t}")
                      for t in range(NT)]
            cc2_out = [dram.tile([P, 1], f32, name=f"cc2_out{t}",
                                 addr_space="Shared") for t in range(NT)]

            # wt1 lifetime: [start .. end of T1p2] — released manually since
            # it overlaps (not nests with) the wt2 pool's lifetime.
            wt1_pool = tc.alloc_tile_pool(name="wt1p", bufs=1, side="right")
            wt1 = wt1_pool.tile([P, KO_1, V1], bf16, name="wt1")
            nc.sync.dma_start(wt1[:], wt1_d[:])  # overlaps P/H phases

            with ExitStack() as ph:
                ph_pool = ph.enter_context(tc.tile_pool(name="ph", bufs=1))
                whead_pool = ph.enter_context(
                    tc.tile_pool(name="whead", bufs=2))
                headout_pool = ph.enter_context(
                    tc.tile_pool(name="headout", bufs=2))

                xT = ph_pool.tile([P, KO_H, T], bf16, name="xT")
                wp1 = ph_pool.tile([P, KO_H, E1], bf16, name="wp1")
                wp2 = ph_pool.tile([P, KO_H, E2], bf16, name="wp2")
                wcl = ph_pool.tile([P, KO_H, 2], bf16, name="wcl")

                nc.sync.dma_start(xT[:], xT_d[:])
                nc.sync.dma_start(wp1[:], wp1_d[:])
                nc.sync.dma_start(wp2[:], wp2_d[:])
                nc.sync.dma_start(wcl[:], wcl_d[:])

                # ---------- Phase P: projections + cluster logits ----------
                # projT[e, t] = sum_h W_proj[e, h] * x[t, h]
                for proj_sb, wp_sb, ko in ((p1T, wp1, KO_1), (p2T, wp2, KO_2)):
                    for e in range(ko):
                        for tc2 in range(2):  # halves of T=1024
                            ps = psum.tile([P, 512], f32, tag="mm")
                            for k in range(KO_H):
                                nc.tensor.matmul(
                                    ps[:],
                                    wp_sb[:, k, e * P:(e + 1) * P],
                                    xT[:, k, tc2 * 512:(tc2 + 1) * 512],
                                    start=(k == 0), stop=(k == KO_H - 1))
                            nc.vector.tensor_copy(
                                proj_sb[:, e, tc2 * 512:(tc2 + 1) * 512],
                                ps[:])
                # cluster logits per token tile: [128, 2]
                for t in range(NT):
                    ps = psum.tile([P, 512], f32, tag="mm")
                    for k in range(KO_H):
                        nc.tensor.matmul(
                            ps[:, :2], xT[:, k, t * P:(t + 1) * P],
                            wcl[:, k, :],
                            start=(k == 0), stop=(k == KO_H - 1))
                    nc.vector.tensor_copy(cl[:, t, :], ps[:, :2])

                # ---------- Phase H: head raw logits ----------
                for c in range(NC_H):
                    wh = whead_pool.tile([P, KO_H, C], bf16, tag="whc")
                    nc.sync.dma_start(wh[:], whead_d[:, :, c * C:(c + 1) * C])
                    ho = headout_pool.tile([P, NT, C], f32, tag="ho")
                    for t in range(NT):
                        ps = psum.tile([P, 512], f32, tag="mm")
                        for k in range(KO_H):
                            nc.tensor.matmul(
                                ps[:, :C], xT[:, k, t * P:(t + 1) * P],
                                wh[:, k, :],
                                start=(k == 0), stop=(k == KO_H - 1))
                        if t % 2 == 0:
                            nc.vector.tensor_copy(ho[:, t, :], ps[:, :C])
                        else:
                            nc.scalar.copy(ho[:, t, :], ps[:, :C])
                    nc.sync.dma_start(out_r[:, :, c * C:(c + 1) * C], ho[:])

            # ph pools closed; wt2 loads during T1 compute.
            with ExitStack() as t1s:
                wt2_pool = t1s.enter_context(
                    tc.tile_pool(name="wt2p", bufs=1))
                wt2 = wt2_pool.tile([P, KO_2, V2], bf16, name="wt2")
                nc.sync.dma_start(wt2[:], wt2_d[:])  # overlaps T1

                # ---------- Phase T1 pass1: tail1 exp-sums ----------
                for t in range(NT):
                    for c in range(NC_1):
                        ps = psum.tile([P, 512], f32, tag="mm")
                        for k in range(KO_1):
                            nc.tensor.matmul(
                                ps[:, :C], p1T[:, k, t * P:(t + 1) * P],
                                wt1[:, k, c * C:(c + 1) * C],
                                start=(k == 0), stop=(k == KO_1 - 1))
                        ex = scratch.tile([P, C], f32, tag="ex")
                        nc.scalar.activation(
                            ex[:], ps[:, :C], Exp,
                            accum_out=s1acc[:, t, c:c + 1])
                    nc.vector.reduce_sum(
                        s1[:, t:t + 1], s1acc[:, t, :],
                        axis=mybir.AxisListType.X)
                nc.sync.dma_start(cc1_in[:], s1[:])
                nc.gpsimd.collective_compute(
                    "AllReduce", mybir.AluOpType.add,
                    replica_groups=rg,
                    ins=[cc1_in[:].opt()], outs=[cc1_out[:].opt()])
                nc.sync.dma_start(g1[:], cc1_out[:])
                lng1 = scratch.tile([P, NT], f32, tag="lng1")
                nc.scalar.activation(lng1[:], g1[:], Ln)
                nc.vector.tensor_sub(out=b1[:], in0=cl[:, :, 0],
                                     in1=lng1[:])

                # ---------- Phase T1 pass2: recompute + bias ----------
                for t in range(NT):
                    oo = outp.tile([P, VOUT // 10], f32, tag="oo")
                    for c in range(NC_1):
                        ps = psum.tile([P, 512], f32, tag="mm")
                        for k in range(KO_1):
                            nc.tensor.matmul(
                                ps[:, :C], p1T[:, k, t * P:(t + 1) * P],
                                wt1[:, k, c * C:(c + 1) * C],
                                start=(k == 0), stop=(k == KO_1 - 1))
                        dst = oo[:, (c % 5) * C:(c % 5 + 1) * C]
                        if c % 2 == 0:
                            nc.scalar.activation(dst, ps[:, :C], Ident,
                                                 bias=b1[:, t:t + 1])
                        else:
                            nc.vector.tensor_scalar_add(
                                dst, ps[:, :C], b1[:, t:t + 1])
                        if c % 5 == 4:
                            nc.sync.dma_start(
                                out_r[:, t,
                                      VH + (c - 4) * C: VH + (c + 1) * C],
                                oo[:])
                            if c + 1 < NC_1:
                                oo = outp.tile([P, VOUT // 10], f32,
                                               tag="oo")

                wt1_pool.release()  # frees 40KB/part for the tail2 staging

                # ---------- Phase T2: tail2, staged + per-tile AR ----------
                with ExitStack() as t2s:
                    stage_pool = t2s.enter_context(
                        tc.tile_pool(name="stage2", bufs=2))
                    small = t2s.enter_context(
                        tc.tile_pool(name="small2", bufs=2))
                    for t in range(NT):
                        stg = stage_pool.tile([P, V2], bf16, tag="stg")
                        s2acc = small.tile([P, NC_2], f32, tag="s2acc")
                        for c in range(NC_2):
                            ps = psum.tile([P, 512], f32, tag="mm")
                            for k in range(KO_2):
                                nc.tensor.matmul(
                                    ps[:, :C],
                                    p2T[:, k, t * P:(t + 1) * P],
                                    wt2[:, k, c * C:(c + 1) * C],
                                    start=(k == 0), stop=(k == KO_2 - 1))
                            ex = scratch.tile([P, C], f32, tag="ex")
                            nc.scalar.activation(
                                ex[:], ps[:, :C], Exp,
                                accum_out=s2acc[:, c:c + 1])
                            nc.vector.tensor_copy(
                                stg[:, c * C:(c + 1) * C], ps[:, :C])
                        s2 = small.tile([P, 1], f32, tag="s2")
                        nc.vector.reduce_sum(s2[:], s2acc[:],
                                             axis=mybir.AxisListType.X)
                        nc.sync.dma_start(cc2_in[t][:], s2[:])
                        nc.gpsimd.collective_compute(
                            "AllReduce", mybir.AluOpType.add,
                            replica_groups=rg,
                            ins=[cc2_in[t][:].opt()],
                            outs=[cc2_out[t][:].opt()])
                        g2 = small.tile([P, 1], f32, tag="g2")
                        nc.sync.dma_start(g2[:], cc2_out[t][:])
                        lng2 = small.tile([P, 1], f32, tag="lng2")
                        nc.scalar.activation(lng2[:], g2[:], Ln)
                        b2 = small.tile([P, 1], f32, tag="b2")
                        nc.vector.tensor_sub(out=b2[:],
                                             in0=cl[:, t, 1:2],
                                             in1=lng2[:])
                        oo = outp.tile([P, VOUT // 10], f32, tag="oo")
                        for c in range(NC_2):
                            src = stg[:, c * C:(c + 1) * C]
                            dst = oo[:, (c % 5) * C:(c % 5 + 1) * C]
                            if c % 2 == 0:
                                nc.scalar.activation(dst, src, Ident,
                                                     bias=b2[:])
                            else:
                                nc.vector.tensor_scalar_add(dst, src,
                                                            b2[:])
                            if c % 5 == 4:
                                nc.sync.dma_start(
                                    out_r[:, t,
                                          VH + V1 + (c - 4) * C:
                                          VH + V1 + (c + 1) * C],
                                    oo[:])
                                if c + 1 < NC_2:
                                    oo = outp.tile([P, VOUT // 10], f32,
                                                   tag="oo")

    nc.compile()
    return nc


def _get_nc():
    if "nc" not in _CACHE:
        _CACHE["nc"] = _build()
    return _CACHE["nc"]


def _prep_inputs(x, W_head, W_proj1, W_tail1, W_proj2, W_tail2):
    import concourse.mybir as mybir
    bf16 = mybir.dt.np(mybir.dt.bfloat16)

    def kxn(w):  # [N, K] weight -> [128, K//128, N] (K on partitions)
        n, k = w.shape
        return np.ascontiguousarray(
            w.T.reshape(k // P, P, n).transpose(1, 0, 2)).astype(bf16)

    x2 = x.reshape(T, H)
    xT = np.ascontiguousarray(
        x2.T.reshape(KO_H, P, T).transpose(1, 0, 2)).astype(bf16)
    wcl = kxn(W_head[20000:20002])
    wp1 = kxn(W_proj1)
    wp2 = kxn(W_proj2)

    in_maps = []
    for i in range(N_CORES):
        in_maps.append({
            "xT": xT,
            "wheadT": kxn(W_head[i * VH:(i + 1) * VH]),
            "wclT": wcl,
            "wp1T": wp1,
            "wp2T": wp2,
            "wt1T": kxn(W_tail1[i * V1:(i + 1) * V1]),
            "wt2T": kxn(W_tail2[i * V2:(i + 1) * V2]),
        })
    return in_maps


def _assemble(outs):
    final = np.empty((T, 200000), dtype=np.float32)
    for i in range(N_CORES):
        o = outs[i]["out"]
        final[:, i * VH:(i + 1) * VH] = o[:, :VH]
        final[:, 20000 + i * V1:20000 + (i + 1) * V1] = o[:, VH:VH + V1]
        final[:, 60000 + i * V2:60000 + (i + 1) * V2] = o[:, VH + V1:]
    return final.reshape(2, 512, 200000)


def _run(inputs, trace=False, tmpdir=None):
    from concourse import bass_utils
    nc = _get_nc()
    in_maps = _prep_inputs(**inputs)
    res = bass_utils.run_bass_kernel_spmd(
        nc, in_maps, core_ids=list(range(N_CORES)), trace=trace,
        tmpdir=tmpdir)
    return _assemble(res.results), res


def kernel(**inputs):
    out, _ = _run(inputs, trace=False)
    return out
